# revision 9
# baseline (speedup 1.0000x reference)
"""Multi-head self-attention with RoPE on 8 Trainium2 NeuronCores.

Sharding: core c = (b, g) with b = c // 4 (batch of 2), g = c % 4 (head
group of 4 heads out of 16). Each core computes Q/K/V projections for its
4 heads on its batch, RoPE, causal attention, producing a context slab
ctxT (256 features x 2048 tokens). A 4-rank AllGather per batch group
assembles the full (1024, 2048) context; each core then applies a
256-column slice of the output projection.

Host-side prep: x is fed pre-transposed per batch; Wq/Wk rows are
permuted per head into [even dims | odd dims] order so RoPE becomes two
elementwise multiplies plus an add against cos/sin tables, with the
"rotate half" partner produced by 32-row SBUF block swaps. Matmuls run
in float32r (single-pass PE mode). The softmax denominator rides the PV
matmul as a per-head ones column appended to V (M=65 outputs).

Self-contained: hardcodes all shapes; builds and compiles the SPMD Bass
program once per process.
"""
import os
import numpy as np

import concourse.bass as bass
import concourse.mybir as mybir
import concourse.tile as tile
from concourse import bacc
from concourse.bass_utils import run_bass_kernel_spmd

B, S, D, H, DK = 2, 2048, 1024, 16, 64
NF = DK // 2            # 32 rotary frequencies
HPC = 4                 # heads per core
GF = HPC * DK           # 256 features per core
NCORES = 8
THETA = 10000.0
F32 = mybir.dt.float32
F32R = mybir.dt.float32r
AF = mybir.ActivationFunctionType

_CACHE: dict = {}


def _emit(nc: bacc.Bacc, phase_limit: int = 3, debug: bool = False):
    xT = nc.dram_tensor("xT", [D, S], F32R, kind="ExternalInput").ap()
    wqT = nc.dram_tensor("wqT", [D, GF], F32R, kind="ExternalInput").ap()
    wkT = nc.dram_tensor("wkT", [D, GF], F32R, kind="ExternalInput").ap()
    wvT = nc.dram_tensor("wvT", [D, GF], F32R, kind="ExternalInput").ap()
    woT = nc.dram_tensor("woT", [D, GF], F32R, kind="ExternalInput").ap()
    cs_d = nc.dram_tensor("cs", [128, S], F32, kind="ExternalInput").ap()
    ss_d = nc.dram_tensor("ss", [128, S], F32, kind="ExternalInput").ap()
    ones_d = nc.dram_tensor("ones", [128, 64], F32R, kind="ExternalInput").ap()
    out_d = nc.dram_tensor("out", [S, GF], F32, kind="ExternalOutput").ap()
    dbg = {}
    if debug:
        for nm in ("dbg_qt0", "dbg_qt1", "dbg_kt0", "dbg_kt1"):
            dbg[nm] = nc.dram_tensor(nm, [128, S], F32, kind="ExternalOutput").ap()
        dbg["dbg_v"] = nc.dram_tensor("dbg_v", [128, (S // 128) * 260], F32, kind="ExternalOutput").ap()
        for h in range(HPC):
            dbg[f"dbg_ctx{h}"] = nc.dram_tensor(f"dbg_ctx{h}", [64, S], F32, kind="ExternalOutput").ap()

    NKT = D // 128       # 8 contraction tiles for projections
    NJ = S // 512        # 4 token 512-blocks
    NQJ = S // 256       # 8 query 256-blocks
    NVT = S // 128       # 16 token 128-blocks (keys for PV / rows for V)

    with tile.TileContext(nc) as tc:
        with (
            tc.tile_pool(name="singles", bufs=1) as singles,
            tc.tile_pool(name="dram", bufs=1, space="DRAM") as dram,
        ):
            # ---- resident tiles ----
            wq_sb = singles.tile([128, NKT, GF], F32R, tag="wq")
            wk_sb = singles.tile([128, NKT, GF], F32R, tag="wk")
            wv_sb = singles.tile([128, NKT, GF], F32R, tag="wv")
            nc.sync.dma_start(out=wq_sb[:], in_=wqT.rearrange("(k p) n -> p k n", p=128))
            nc.sync.dma_start(out=wk_sb[:], in_=wkT.rearrange("(k p) n -> p k n", p=128))
            nc.sync.dma_start(out=wv_sb[:], in_=wvT.rearrange("(k p) n -> p k n", p=128))
            cs_sb = singles.tile([128, S], F32, tag="cs")
            ss_sb = singles.tile([128, S], F32, tag="ss")
            nc.sync.dma_start(out=cs_sb[:], in_=cs_d[:])
            nc.sync.dma_start(out=ss_sb[:], in_=ss_d[:])
            ones_sb = singles.tile([128, 64], F32R, tag="ones")
            nc.sync.dma_start(out=ones_sb[:], in_=ones_d[:])

            # roped Q^T / K^T: 2 tiles each, rows = [headA(64) | headB(64)],
            # within each head block [x0(32) | x1(32)]
            qt = [singles.tile([128, S], F32R, tag=f"qt{m}", name=f"qt{m}") for m in range(2)]
            kt = [singles.tile([128, S], F32R, tag=f"kt{m}", name=f"kt{m}") for m in range(2)]
            # V with per-head ones column: head h occupies cols 65h..65h+63,
            # col 65h+64 is 1.0 (softmax denominator rides the PV matmul)
            v_sb = singles.tile([128, NVT, 4 * 65], F32R, tag="v")
            nc.vector.tensor_copy(
                v_sb.rearrange("p t (h e) -> p t h e", h=4)[:, :, :, 64:65],
                ones_sb.rearrange("p (t h) -> p t h", t=NVT)[:, :, :, None])
            # context output per head (64 rows each; sums row stays in psum)
            ctx_sb = [singles.tile([64, S], F32R, tag=f"ctx{h}", name=f"ctx{h}") for h in range(HPC)]

            # ---- phase 1: QKV projections + RoPE ----
            with (
                tc.tile_pool(name="xin", bufs=12) as xin,
                tc.tile_pool(name="qkraw", bufs=4) as qkraw,
                tc.tile_pool(name="ropetmp", bufs=4) as ropetmp,
                tc.tile_pool(name="ps_qk", bufs=2, space="PSUM") as ps_qk,
                tc.tile_pool(name="ps_v", bufs=2, space="PSUM") as ps_v,
            ):
                for j in range(NJ):
                    csl = slice(512 * j, 512 * (j + 1))
                    xts = []
                    for k in range(NKT):
                        xt_ = xin.tile([128, 512], F32R)
                        nc.sync.dma_start(out=xt_[:], in_=xT[128 * k:128 * (k + 1), csl])
                        xts.append(xt_)
                    # Q^T and K^T tiles: out (128 qdim, 512 tok)
                    for w_sb, raw_dst in ((wq_sb, qt), (wk_sb, kt)):
                        for m in range(2):
                            pq = ps_qk.tile([128, 512], F32)
                            for k in range(NKT):
                                nc.tensor.matmul(
                                    pq[:], w_sb[:, k, 128 * m:128 * (m + 1)], xts[k][:],
                                    start=(k == 0), stop=(k == NKT - 1))
                            raw = qkraw.tile([128, 512], F32)
                            nc.scalar.copy(out=raw[:], in_=pq[:])
                            # rope: dst = raw*cs + swap(raw)*ss
                            sw = ropetmp.tile([128, 512], F32, tag="sw")
                            for blk in range(2):
                                nc.sync.dma_start(out=sw[64 * blk:64 * blk + 32, :],
                                                  in_=raw[64 * blk + 32:64 * blk + 64, :])
                                nc.sync.dma_start(out=sw[64 * blk + 32:64 * blk + 64, :],
                                                  in_=raw[64 * blk:64 * blk + 32, :])
                            t1 = ropetmp.tile([128, 512], F32, tag="t1")
                            nc.vector.tensor_mul(t1[:], raw[:], cs_sb[:, csl])
                            nc.vector.tensor_mul(sw[:], sw[:], ss_sb[:, csl])
                            nc.vector.tensor_add(raw_dst[m][:, csl], t1[:], sw[:])
                    # V tiles: out (128 tok, 256 dims) scattered into 65-stride layout
                    for s_ in range(4):
                        vt = 4 * j + s_
                        pv = ps_v.tile([128, GF], F32)
                        for k in range(NKT):
                            nc.tensor.matmul(
                                pv[:], xts[k][:, 128 * s_:128 * (s_ + 1)], wv_sb[:, k, :],
                                start=(k == 0), stop=(k == NKT - 1))
                        dst = v_sb[:, vt, :].rearrange("p (h e) -> p h e", h=4)[:, :, 0:64]
                        nc.vector.tensor_copy(dst, pv[:].rearrange("p (h e) -> p h e", h=4))

            if debug:
                for m in range(2):
                    nc.sync.dma_start(out=dbg[f"dbg_qt{m}"][:], in_=qt[m][:].bitcast(F32))
                    nc.sync.dma_start(out=dbg[f"dbg_kt{m}"][:], in_=kt[m][:].bitcast(F32))
                nc.sync.dma_start(out=dbg["dbg_v"][:],
                                  in_=v_sb.rearrange("p t e -> p (t e)").bitcast(F32))
            if phase_limit < 2:
                return

            # ---- phase 2: attention per (pair p, q-block qj) ----
            inv_sqrt_dk = float(1.0 / np.sqrt(DK))
            with (
                tc.tile_pool(name="probs", bufs=3) as probspool,
                tc.tile_pool(name="recips", bufs=2) as recips,
                tc.tile_pool(name="ctxu", bufs=3) as ctxupool,
                tc.tile_pool(name="ps_sc", bufs=2, space="PSUM") as ps_sc,
                tc.tile_pool(name="ps_ctx", bufs=3, space="PSUM") as ps_ctx,
                tc.tile_pool(name="ps_bc", bufs=1, space="PSUM") as ps_bc,
            ):
                for p in range(2):
                    for qj in range(NQJ):
                        qsl = slice(256 * qj, 256 * (qj + 1))
                        nch = qj + 1                     # chunks of 2 key-tiles
                        pctx = [ps_ctx.tile([65, 256], F32, tag="ctx", name="pctx") for _ in range(2)]
                        for ch in range(nch):
                            psc = ps_sc.tile([128, 1024], F32)
                            for ki in range(2):
                                ktile = 2 * ch + ki
                                for hh in range(2):
                                    rsl = slice(64 * hh, 64 * (hh + 1))
                                    nc.tensor.matmul(
                                        psc[:, 512 * hh + 256 * ki:512 * hh + 256 * ki + 256],
                                        kt[p][rsl, 128 * ktile:128 * (ktile + 1)],
                                        qt[p][rsl, qsl],
                                        start=True, stop=True)
                            probs = probspool.tile([128, 1024], F32R)
                            nc.scalar.activation(out=probs[:], in_=psc[:],
                                                 func=AF.Exp, scale=inv_sqrt_dk)
                            if ch == nch - 1:  # diagonal: zero where key > query
                                for ki in range(2):
                                    ktile = 2 * ch + ki
                                    for hh in range(2):
                                        sl = probs[:, 512 * hh + 256 * ki:512 * hh + 256 * ki + 256]
                                        nc.gpsimd.affine_select(
                                            out=sl, in_=sl,
                                            compare_op=mybir.AluOpType.is_ge,
                                            fill=0.0, base=256 * qj - 128 * ktile,
                                            pattern=[[1, 256]], channel_multiplier=-1)
                            for ki in range(2):
                                ktile = 2 * ch + ki
                                for hh in range(2):
                                    h65 = 65 * (2 * p + hh)
                                    nc.tensor.matmul(
                                        pctx[hh][:],
                                        v_sb[:, ktile, h65:h65 + 65],
                                        probs[:, 512 * hh + 256 * ki:512 * hh + 256 * ki + 256],
                                        start=(ch == 0 and ki == 0),
                                        stop=(ch == nch - 1 and ki == 1))
                        recip = recips.tile([128, 512], F32R)
                        for hh in range(2):
                            ctxu = ctxupool.tile([65, 256], F32, tag="ctxu", name="ctxu")
                            nc.scalar.copy(out=ctxu[:], in_=pctx[hh][:])
                            with nc.allow_low_precision(reason="f32r recip feeds f32r matmul"):
                                nc.vector.reciprocal(out=recip[64:65, 256 * hh:256 * (hh + 1)],
                                                     in_=ctxu[64:65, :])
                            pbc = ps_bc.tile([64, 256], F32, tag="bc")
                            nc.tensor.matmul(
                                pbc[:], ones_sb[64:65, 0:64],
                                recip[64:65, 256 * hh:256 * (hh + 1)],
                                start=True, stop=True)
                            nc.vector.tensor_mul(ctx_sb[2 * p + hh][:, qsl],
                                                 ctxu[0:64, :], pbc[:])

            if debug:
                for h in range(HPC):
                    nc.sync.dma_start(out=dbg[f"dbg_ctx{h}"][:], in_=ctx_sb[h][:].bitcast(F32))
            if phase_limit < 3:
                return

            # ---- phase 3: AllGather + output projection ----
            ag_in = dram.tile([GF, S], F32R)
            ag_out = dram.tile([D, S], F32R)
            for h in range(HPC):
                nc.sync.dma_start(out=ag_in[64 * h:64 * (h + 1), :], in_=ctx_sb[h][:])
            nc.gpsimd.collective_compute(
                "AllGather", mybir.AluOpType.bypass,
                replica_groups=[[0, 1, 2, 3], [4, 5, 6, 7]],
                ins=[ag_in.opt()], outs=[ag_out.opt()])

            with (
                tc.tile_pool(name="ph3", bufs=1) as ph3,
                tc.tile_pool(name="outsb", bufs=3) as outsb,
                tc.tile_pool(name="ps_o", bufs=3, space="PSUM") as ps_o,
            ):
                wo_sb = ph3.tile([128, NKT, GF], F32R, tag="wo")
                nc.sync.dma_start(out=wo_sb[:], in_=woT.rearrange("(k p) n -> p k n", p=128))
                for half in range(2):
                    hsl = slice(1024 * half, 1024 * (half + 1))
                    ag_sb = []
                    for k in range(NKT):
                        t = ph3.tile([128, 1024], F32R, tag=f"ag{k}")
                        nc.sync.dma_start(out=t[:], in_=ag_out[128 * k:128 * (k + 1), hsl])
                        ag_sb.append(t)
                    for mt8 in range(8):
                        mt = 8 * half + mt8
                        po = ps_o.tile([128, GF], F32)
                        for k in range(NKT):
                            nc.tensor.matmul(
                                po[:], ag_sb[k][:, 128 * mt8:128 * (mt8 + 1)], wo_sb[:, k, :],
                                start=(k == 0), stop=(k == NKT - 1))
                        ot = outsb.tile([128, GF], F32)
                        nc.scalar.copy(out=ot[:], in_=po[:])
                        nc.sync.dma_start(out=out_d[128 * mt:128 * (mt + 1), :], in_=ot[:])


def _build():
    nc = bacc.Bacc("TRN2", target_bir_lowering=False, debug=False, num_devices=NCORES)
    _emit(nc)
    nc.compile()
    return nc


def _perm_rows(g: int) -> np.ndarray:
    rows = []
    for l in range(HPC):
        h = HPC * g + l
        rows += [DK * h + d for d in range(0, DK, 2)]
        rows += [DK * h + d for d in range(1, DK, 2)]
    return np.asarray(rows)


def kernel(x, token_positions, Wq, Wk, Wv, Wo):
    x = np.asarray(x, dtype=np.float32)
    Wq = np.asarray(Wq, dtype=np.float32)
    Wk = np.asarray(Wk, dtype=np.float32)
    Wv = np.asarray(Wv, dtype=np.float32)
    Wo = np.asarray(Wo, dtype=np.float32)
    pos = np.asarray(token_positions).astype(np.float64)

    if "nc" not in _CACHE:
        _CACHE["nc"] = _build()
    nc = _CACHE["nc"]

    inv_freq = np.exp(np.arange(0, DK, 2, dtype=np.float32) * (-np.log(THETA) / DK)).astype(np.float64)
    ang = pos[:, None] * inv_freq[None, :]              # (S, 32)
    cos_t = np.cos(ang).astype(np.float32).T            # (32, S)
    sin_t = np.sin(ang).astype(np.float32).T
    fi = np.arange(128) % NF
    half = (np.arange(128) // NF) % 2
    CS = np.ascontiguousarray(cos_t[fi, :])
    SS = np.ascontiguousarray(np.where(half[:, None] == 0, -sin_t[fi, :], sin_t[fi, :]))
    ONES = np.ones((128, 64), dtype=np.float32)

    in_maps = []
    for c in range(NCORES):
        b, g = divmod(c, 4)
        pr = _perm_rows(g)
        in_maps.append({
            "xT": np.ascontiguousarray(x[b].T),
            "wqT": np.ascontiguousarray(Wq[pr].T),
            "wkT": np.ascontiguousarray(Wk[pr].T),
            "wvT": np.ascontiguousarray(Wv[GF * g:GF * (g + 1)].T),
            "woT": np.ascontiguousarray(Wo[GF * g:GF * (g + 1)].T),
            "cs": CS, "ss": SS, "ones": ONES,
        })

    trace = os.environ.get("KERNEL_TRACE", "0") == "1"
    res = run_bass_kernel_spmd(nc, in_maps, list(range(NCORES)), trace=trace)
    _CACHE["last_result"] = res

    out = np.empty((B, S, D), dtype=np.float32)
    for c in range(NCORES):
        b, g = divmod(c, 4)
        out[b, :, GF * g:GF * (g + 1)] = res.results[c]["out"]
    return out


# revision 13
# speedup vs baseline: 1.4817x; 1.4817x over previous
"""Multi-head self-attention with RoPE on 8 Trainium2 NeuronCores.

Sharding: core c = (b, g) with b = c // 4 (batch of 2), g = c % 4 (head
group of 4 heads out of 16). Each core computes Q/K/V projections for its
4 heads on its batch, RoPE, causal attention, producing a context slab
ctxT (256 features x 2048 tokens). A 4-rank AllGather per batch group
assembles the full (1024, 2048) context; each core then applies a
256-column slice of the output projection.

Host-side prep: x is fed pre-transposed per batch; Wq/Wk rows are
permuted per head into [even dims | odd dims] order so RoPE becomes two
elementwise multiplies plus an add against cos/sin tables, with the
"rotate half" partner produced by 32-row SBUF block swaps. Matmuls run
in float32r (single-pass PE mode). The softmax denominator rides the PV
matmul as a per-head ones column appended to V (M=65 outputs).

Self-contained: hardcodes all shapes; builds and compiles the SPMD Bass
program once per process.
"""
import os
import numpy as np

import concourse.bass as bass
import concourse.mybir as mybir
import concourse.tile as tile
from concourse import bacc
from concourse.bass_utils import run_bass_kernel_spmd

B, S, D, H, DK = 2, 2048, 1024, 16, 64
NF = DK // 2            # 32 rotary frequencies
HPC = 4                 # heads per core
GF = HPC * DK           # 256 features per core
NCORES = 8
THETA = 10000.0
F32 = mybir.dt.float32
F32R = mybir.dt.float32r
AF = mybir.ActivationFunctionType

_CACHE: dict = {}


def _emit(nc: bacc.Bacc, phase_limit: int = 3, debug: bool = False):
    xT = nc.dram_tensor("xT", [D, S], F32R, kind="ExternalInput").ap()
    wqT = nc.dram_tensor("wqT", [D, GF], F32R, kind="ExternalInput").ap()
    wkT = nc.dram_tensor("wkT", [D, GF], F32R, kind="ExternalInput").ap()
    wvT = nc.dram_tensor("wvT", [D, GF], F32R, kind="ExternalInput").ap()
    woT = nc.dram_tensor("woT", [D, GF], F32R, kind="ExternalInput").ap()
    cs_d = nc.dram_tensor("cs", [128, S], F32, kind="ExternalInput").ap()
    ss_d = nc.dram_tensor("ss", [128, S], F32, kind="ExternalInput").ap()
    ones_d = nc.dram_tensor("ones", [128, 64], F32R, kind="ExternalInput").ap()
    out_d = nc.dram_tensor("out", [GF, S], F32, kind="ExternalOutput").ap()
    dbg = {}
    if debug:
        for nm in ("dbg_qt0", "dbg_qt1", "dbg_kt0", "dbg_kt1"):
            dbg[nm] = nc.dram_tensor(nm, [128, S], F32, kind="ExternalOutput").ap()
        dbg["dbg_v"] = nc.dram_tensor("dbg_v", [128, (S // 128) * 260], F32, kind="ExternalOutput").ap()
        for h in range(HPC):
            dbg[f"dbg_ctx{h}"] = nc.dram_tensor(f"dbg_ctx{h}", [64, S], F32, kind="ExternalOutput").ap()

    NKT = D // 128       # 8 contraction tiles for projections
    NJ = S // 512        # 4 token 512-blocks
    NQJ = S // 256       # 8 query 256-blocks
    NVT = S // 128       # 16 token 128-blocks (keys for PV / rows for V)

    with tile.TileContext(nc) as tc:
        with (
            tc.tile_pool(name="singles", bufs=1) as singles,
            tc.tile_pool(name="dram", bufs=1, space="DRAM") as dram,
        ):
            # ---- resident tiles ----
            wq_sb = singles.tile([128, NKT, GF], F32R, tag="wq")
            wk_sb = singles.tile([128, NKT, GF], F32R, tag="wk")
            wv_sb = singles.tile([128, NKT, GF], F32R, tag="wv")
            nc.sync.dma_start(out=wq_sb[:], in_=wqT.rearrange("(k p) n -> p k n", p=128))
            nc.sync.dma_start(out=wk_sb[:], in_=wkT.rearrange("(k p) n -> p k n", p=128))
            nc.sync.dma_start(out=wv_sb[:], in_=wvT.rearrange("(k p) n -> p k n", p=128))
            cs_sb = singles.tile([128, S], F32, tag="cs")
            ss_sb = singles.tile([128, S], F32, tag="ss")
            nc.sync.dma_start(out=cs_sb[:], in_=cs_d[:])
            nc.sync.dma_start(out=ss_sb[:], in_=ss_d[:])
            ones_sb = singles.tile([128, 64], F32R, tag="ones")
            nc.sync.dma_start(out=ones_sb[:], in_=ones_d[:])

            # roped Q^T / K^T: 2 tiles each, rows = [headA(64) | headB(64)],
            # within each head block [x0(32) | x1(32)]
            qt = [singles.tile([128, S], F32R, tag=f"qt{m}", name=f"qt{m}") for m in range(2)]
            kt = [singles.tile([128, S], F32R, tag=f"kt{m}", name=f"kt{m}") for m in range(2)]
            # V with per-head ones column: head h occupies cols 65h..65h+63,
            # col 65h+64 is 1.0 (softmax denominator rides the PV matmul)
            v_sb = singles.tile([128, NVT, 4 * 65], F32R, tag="v")
            nc.vector.tensor_copy(
                v_sb.rearrange("p t (h e) -> p t h e", h=4)[:, :, :, 64:65],
                ones_sb.rearrange("p (t h) -> p t h", t=NVT)[:, :, :, None])
            # context output per head (64 rows each; sums row stays in psum)
            ctx_sb = [singles.tile([64, S], F32R, tag=f"ctx{h}", name=f"ctx{h}") for h in range(HPC)]

            # ---- phase 1: QKV projections + RoPE ----
            with (
                tc.tile_pool(name="xin", bufs=12) as xin,
                tc.tile_pool(name="qkraw", bufs=4) as qkraw,
                tc.tile_pool(name="ropetmp", bufs=4) as ropetmp,
                tc.tile_pool(name="ps_qk", bufs=2, space="PSUM") as ps_qk,
                tc.tile_pool(name="ps_v", bufs=2, space="PSUM") as ps_v,
            ):
                for j in range(NJ):
                    csl = slice(512 * j, 512 * (j + 1))
                    xts = []
                    for k in range(NKT):
                        xt_ = xin.tile([128, 512], F32R)
                        nc.sync.dma_start(out=xt_[:], in_=xT[128 * k:128 * (k + 1), csl])
                        xts.append(xt_)
                    # Q^T and K^T tiles: out (128 qdim, 512 tok)
                    for w_sb, raw_dst in ((wq_sb, qt), (wk_sb, kt)):
                        for m in range(2):
                            pq = ps_qk.tile([128, 512], F32)
                            for k in range(NKT):
                                nc.tensor.matmul(
                                    pq[:], w_sb[:, k, 128 * m:128 * (m + 1)], xts[k][:],
                                    start=(k == 0), stop=(k == NKT - 1))
                            raw = qkraw.tile([128, 512], F32)
                            nc.scalar.copy(out=raw[:], in_=pq[:])
                            # rope: dst = raw*cs + swap(raw)*ss
                            sw = ropetmp.tile([128, 512], F32, tag="sw")
                            for blk in range(2):
                                nc.sync.dma_start(out=sw[64 * blk:64 * blk + 32, :],
                                                  in_=raw[64 * blk + 32:64 * blk + 64, :])
                                nc.sync.dma_start(out=sw[64 * blk + 32:64 * blk + 64, :],
                                                  in_=raw[64 * blk:64 * blk + 32, :])
                            t1 = ropetmp.tile([128, 512], F32, tag="t1")
                            nc.vector.tensor_mul(t1[:], raw[:], cs_sb[:, csl])
                            nc.vector.tensor_mul(sw[:], sw[:], ss_sb[:, csl])
                            nc.vector.tensor_add(raw_dst[m][:, csl], t1[:], sw[:])
                    # V tiles: out (128 tok, 256 dims) scattered into 65-stride layout
                    for s_ in range(4):
                        vt = 4 * j + s_
                        pv = ps_v.tile([128, GF], F32)
                        for k in range(NKT):
                            nc.tensor.matmul(
                                pv[:], xts[k][:, 128 * s_:128 * (s_ + 1)], wv_sb[:, k, :],
                                start=(k == 0), stop=(k == NKT - 1))
                        dst = v_sb[:, vt, :].rearrange("p (h e) -> p h e", h=4)[:, :, 0:64]
                        nc.vector.tensor_copy(dst, pv[:].rearrange("p (h e) -> p h e", h=4))

            if debug:
                for m in range(2):
                    nc.sync.dma_start(out=dbg[f"dbg_qt{m}"][:], in_=qt[m][:].bitcast(F32))
                    nc.sync.dma_start(out=dbg[f"dbg_kt{m}"][:], in_=kt[m][:].bitcast(F32))
                nc.sync.dma_start(out=dbg["dbg_v"][:],
                                  in_=v_sb.rearrange("p t e -> p (t e)").bitcast(F32))
            if phase_limit < 2:
                return

            # ---- phase 2: attention per (q-block qj of 512, pair p) ----
            # scoresT psum chunk = 1 key-tile: [headA (128,512) bank | headB bank]
            # AllGather fires per q-block as soon as both pairs finish it.
            inv_sqrt_dk = float(1.0 / np.sqrt(DK))
            NQB = S // 512                       # 4 q-blocks
            ag_in = dram.tile([NQB, GF, 512], F32R)
            ag_out = dram.tile([NQB, D, 512], F32R)
            with (
                tc.tile_pool(name="probs", bufs=3) as probspool,
                tc.tile_pool(name="recips", bufs=2) as recips,
                tc.tile_pool(name="ctxu", bufs=3) as ctxupool,
                tc.tile_pool(name="ps_sc", bufs=2, space="PSUM") as ps_sc,
                tc.tile_pool(name="ps_ctx", bufs=3, space="PSUM") as ps_ctx,
                tc.tile_pool(name="ps_bc", bufs=1, space="PSUM") as ps_bc,
            ):
                for qj in range(NQB):
                    qsl = slice(512 * qj, 512 * (qj + 1))
                    nch = 4 * (qj + 1)               # key-tiles (= chunks)
                    for p in range(2):
                        pctx = [ps_ctx.tile([65, 512], F32, tag="ctx", name="pctx")
                                for _ in range(2)]
                        for ch in range(nch):
                            psc = ps_sc.tile([128, 1024], F32)
                            for hh in range(2):
                                rsl = slice(64 * hh, 64 * (hh + 1))
                                nc.tensor.matmul(
                                    psc[:, 512 * hh:512 * hh + 512],
                                    kt[p][rsl, 128 * ch:128 * (ch + 1)],
                                    qt[p][rsl, qsl],
                                    start=True, stop=True)
                            probs = probspool.tile([128, 1024], F32R)
                            nc.scalar.activation(out=probs[:], in_=psc[:],
                                                 func=AF.Exp, scale=inv_sqrt_dk)
                            if ch >= 4 * qj:  # diagonal: zero where key > query
                                for hh in range(2):
                                    sl = probs[:, 512 * hh:512 * hh + 512]
                                    nc.gpsimd.affine_select(
                                        out=sl, in_=sl,
                                        compare_op=mybir.AluOpType.is_ge,
                                        fill=0.0, base=512 * qj - 128 * ch,
                                        pattern=[[1, 512]], channel_multiplier=-1)
                            for hh in range(2):
                                h65 = 65 * (2 * p + hh)
                                nc.tensor.matmul(
                                    pctx[hh][:],
                                    v_sb[:, ch, h65:h65 + 65],
                                    probs[:, 512 * hh:512 * hh + 512],
                                    start=(ch == 0), stop=(ch == nch - 1))
                        recip = recips.tile([128, 1024], F32, tag="recip", name="recip")
                        recipr = recips.tile([128, 1024], F32R, tag="recipr", name="recipr")
                        ctxus = []
                        for hh in range(2):
                            ctxu = ctxupool.tile([65, 512], F32, tag="ctxu", name="ctxu")
                            nc.scalar.copy(out=ctxu[:], in_=pctx[hh][:])
                            nc.vector.reciprocal_approx_fast(
                                out=recip[0:65, 512 * hh:512 * (hh + 1)],
                                in_=ctxu[:])
                            ctxus.append(ctxu)
                        nc.vector.tensor_copy(recipr[64:65, :], recip[64:65, :])
                        for hh in range(2):
                            pbc = ps_bc.tile([64, 512], F32, tag="bc")
                            nc.tensor.matmul(
                                pbc[:], ones_sb[64:65, 0:64],
                                recipr[64:65, 512 * hh:512 * (hh + 1)],
                                start=True, stop=True)
                            nc.vector.tensor_mul(ctx_sb[2 * p + hh][:, qsl],
                                                 ctxus[hh][0:64, :], pbc[:])
                    # ship this q-block's context and gather it across the group
                    if phase_limit < 3:
                        continue
                    for h in range(HPC):
                        nc.sync.dma_start(out=ag_in[qj, 64 * h:64 * (h + 1), :],
                                          in_=ctx_sb[h][:, qsl])
                    nc.gpsimd.collective_compute(
                        "AllGather", mybir.AluOpType.bypass,
                        replica_groups=[[0, 1, 2, 3], [4, 5, 6, 7]],
                        ins=[ag_in[qj].opt()], outs=[ag_out[qj].opt()])

            if debug:
                for h in range(HPC):
                    nc.sync.dma_start(out=dbg[f"dbg_ctx{h}"][:], in_=ctx_sb[h][:].bitcast(F32))
            if phase_limit < 3:
                return

            # ---- phase 3: output projection (transposed), per q-block ----
            with (
                tc.tile_pool(name="ph3", bufs=1) as ph3,
                tc.tile_pool(name="agsb", bufs=2) as agsb,
                tc.tile_pool(name="outsb", bufs=3) as outsb,
                tc.tile_pool(name="ps_o", bufs=3, space="PSUM") as ps_o,
            ):
                wo_sb = ph3.tile([128, NKT, GF], F32R, tag="wo")
                nc.sync.dma_start(out=wo_sb[:], in_=woT.rearrange("(k p) n -> p k n", p=128))
                for qj in range(NQB):
                    ag_sb = []
                    for k in range(NKT):
                        t = agsb.tile([128, 512], F32R, tag=f"ag{k}", name=f"ag{k}")
                        nc.sync.dma_start(out=t[:], in_=ag_out[qj, 128 * k:128 * (k + 1), :])
                        ag_sb.append(t)
                    for m in range(2):
                        po = ps_o.tile([128, 512], F32)
                        for k in range(NKT):
                            nc.tensor.matmul(
                                po[:], wo_sb[:, k, 128 * m:128 * (m + 1)], ag_sb[k][:],
                                start=(k == 0), stop=(k == NKT - 1))
                        ot = outsb.tile([128, 512], F32)
                        nc.scalar.copy(out=ot[:], in_=po[:])
                        nc.sync.dma_start(out=out_d[128 * m:128 * (m + 1), 512 * qj:512 * (qj + 1)],
                                          in_=ot[:])


def _build():
    nc = bacc.Bacc("TRN2", target_bir_lowering=False, debug=False, num_devices=NCORES)
    _emit(nc)
    nc.compile()
    return nc


def _perm_rows(g: int) -> np.ndarray:
    rows = []
    for l in range(HPC):
        h = HPC * g + l
        rows += [DK * h + d for d in range(0, DK, 2)]
        rows += [DK * h + d for d in range(1, DK, 2)]
    return np.asarray(rows)


def kernel(x, token_positions, Wq, Wk, Wv, Wo):
    x = np.asarray(x, dtype=np.float32)
    Wq = np.asarray(Wq, dtype=np.float32)
    Wk = np.asarray(Wk, dtype=np.float32)
    Wv = np.asarray(Wv, dtype=np.float32)
    Wo = np.asarray(Wo, dtype=np.float32)
    pos = np.asarray(token_positions).astype(np.float64)

    if "nc" not in _CACHE:
        _CACHE["nc"] = _build()
    nc = _CACHE["nc"]

    inv_freq = np.exp(np.arange(0, DK, 2, dtype=np.float32) * (-np.log(THETA) / DK)).astype(np.float64)
    ang = pos[:, None] * inv_freq[None, :]              # (S, 32)
    cos_t = np.cos(ang).astype(np.float32).T            # (32, S)
    sin_t = np.sin(ang).astype(np.float32).T
    fi = np.arange(128) % NF
    half = (np.arange(128) // NF) % 2
    CS = np.ascontiguousarray(cos_t[fi, :])
    SS = np.ascontiguousarray(np.where(half[:, None] == 0, -sin_t[fi, :], sin_t[fi, :]))
    ONES = np.ones((128, 64), dtype=np.float32)

    in_maps = []
    for c in range(NCORES):
        b, g = divmod(c, 4)
        pr = _perm_rows(g)
        in_maps.append({
            "xT": np.ascontiguousarray(x[b].T),
            "wqT": np.ascontiguousarray(Wq[pr].T),
            "wkT": np.ascontiguousarray(Wk[pr].T),
            "wvT": np.ascontiguousarray(Wv[GF * g:GF * (g + 1)].T),
            "woT": np.ascontiguousarray(Wo[GF * g:GF * (g + 1)].T),
            "cs": CS, "ss": SS, "ones": ONES,
        })

    trace = os.environ.get("KERNEL_TRACE", "0") == "1"
    res = run_bass_kernel_spmd(nc, in_maps, list(range(NCORES)), trace=trace)
    _CACHE["last_result"] = res

    out = np.empty((B, S, D), dtype=np.float32)
    for c in range(NCORES):
        b, g = divmod(c, 4)
        out[b, :, GF * g:GF * (g + 1)] = res.results[c]["out"].T
    return out
